# revision 16
# baseline (speedup 1.0000x reference)
"""GCN edge-prediction kernel for 8 trn2 NeuronCores (Bass/Tile).

Math (per GCNConv layer, PyG semantics with self-loops + symmetric norm):
    h = x @ W;  htil = dinv * h  (row scale)
    out[d] = dinv[d] * sum_{e: s->d, incl self} htil[s] + b
Implemented as:
  - node shard of 6250 rows per core; per-layer bf16 node table AllGathered.
    Tables are stored in PERMUTED row order (node n -> row
    core*6272 + (local%128)*49 + local//128) so the per-layer shard emit is
    ONE contiguous 1.6MB DMA from the node-major SBUF tile instead of ~6k
    256-byte descriptors; gathers just relabel indices host-side.
  - per dst-block (128 nodes) edge chunks of 128 edges, each chunk pure in
    one table half (dma_gather idxs are int16; table > 32k rows); chunk
    order = all lower-half chunks (block-major), then all upper-half chunks,
    so gather batches of up to GB chunks per InstDMAGatherAnt stay
    half-pure; scatter-sum via PE matmul with a one-hot indicator carrying
    dinv[dst]
  - decode: labels permuted into (a_half, b_half) groups; per group, batched
    dma_gather of z rows for both endpoints, one wide mul + 3D-AP reduce per
    batch on DVE; host inverse-permutes the logits
"""
import os
import sys

sys.path.insert(0, "/opt/trn_rl_repo")

import numpy as np
import ml_dtypes

import concourse.bass as bass
import concourse.bacc as bacc
import concourse.mybir as mybir
import concourse.tile as tile
from concourse import library_config
from concourse.bass_utils import run_bass_kernel_spmd

NC = 8
P = 128
GB = int(os.environ.get('GCN_GB', '8'))       # chunks per dma_gather
WW = int(os.environ.get('GCN_WW', '512'))     # weight-matmul moving width
XBF16 = bool(int(os.environ.get('GCN_XBF16', '1')))
WBF16 = bool(int(os.environ.get('GCN_WBF16', '1')))
ICOLS = GB * P // 16                          # idx cols per batch slot


def _prow(n, sh, nb):
    """Node id -> permuted table row (see module docstring)."""
    c, u = n // sh, n % sh
    return c * (nb * P) + (u % P) * nb + u // P


def _wrap_idx(flat):
    """int16 idx list -> wrapped [P, ceil(len/16)] with 8 replicated groups."""
    n = len(flat)
    cols = (n + 15) // 16
    out = np.zeros((P, cols), np.int16)
    j = np.arange(n)
    for g in range(8):
        out[g * 16 + (j % 16), j // 16] = flat
    return out


def _batch_chunks(tags):
    """Greedy batches of <=GB consecutive chunks sharing a tag.

    Returns list of (lo, hi, tag) and per-chunk (batch_idx, pos)."""
    batches = []
    chunk_pos = np.zeros((len(tags), 2), np.int64)
    i = 0
    while i < len(tags):
        t = tags[i]
        j = i
        while j < len(tags) and j - i < GB and tags[j] == t:
            j += 1
        for c in range(i, j):
            chunk_pos[c] = (len(batches), c - i)
        batches.append((i, j, int(t)))
        i = j
    return batches, chunk_pos


def _build_plan(n_nodes, edge_index, dinv):
    """Host-side partitioning: half-pure chunks, A-region then B-region."""
    sh = n_nodes // NC
    nb = (sh + P - 1) // P
    phalf = NC * nb * P // 2
    src, dst = edge_index[0].astype(np.int64), edge_index[1].astype(np.int64)

    core = dst // sh
    blk = (dst % sh) // P
    dl = (dst % sh) % P
    srow = _prow(src, sh, nb)
    half = (srow >= phalf).astype(np.int64)

    counts = np.zeros((NC, nb, 2), np.int64)
    np.add.at(counts, (core, blk, half), 1)
    kb2 = -(-counts.max(axis=0) // P)           # [nb, 2] chunks (can be 0)
    nchA = int(kb2[:, 0].sum())
    nch = nchA + int(kb2[:, 1].sum())
    chunk_start = np.zeros((nb, 2), np.int64)
    chunk_start[:, 0] = np.cumsum(kb2[:, 0]) - kb2[:, 0]
    chunk_start[:, 1] = nchA + np.cumsum(kb2[:, 1]) - kb2[:, 1]
    chunk_half = np.zeros(nch, np.int64)
    chunk_half[nchA:] = 1
    batches, chunk_pos = _batch_chunks(chunk_half)

    idxrel = np.zeros((NC, P, nch), np.int16)
    ind = np.zeros((NC, P, nch * P), np.float32)
    order = np.lexsort((dl, half, blk, core))
    srow_s, blk_s, dl_s, dst_s = srow[order], blk[order], dl[order], dst[order]
    core_s, half_s = core[order], half[order]
    key = (core_s * nb + blk_s) * 2 + half_s
    bounds = np.searchsorted(key, np.arange(NC * nb * 2 + 1) - 0.5)
    for c in range(NC):
        for b in range(nb):
            for h in range(2):
                k = (c * nb + b) * 2 + h
                lo, hi = bounds[k], bounds[k + 1]
                if hi == lo:
                    continue
                slot = np.arange(hi - lo)
                ch = chunk_start[b, h] + slot // P
                lane = slot % P
                idxrel[c, lane, ch] = (srow_s[lo:hi] - h * phalf).astype(np.int16)
                ind[c, lane, ch * P + dl_s[lo:hi]] = dinv[dst_s[lo:hi]]
    ind = ind.astype(ml_dtypes.bfloat16)

    agg_idx = np.zeros((NC, P, len(batches) * ICOLS), np.int16)
    for c in range(NC):
        for k, (lo, hi, h) in enumerate(batches):
            flat = idxrel[c][:, lo:hi].T.reshape(-1)   # chunk-major, lane-minor
            w = _wrap_idx(flat)
            agg_idx[c, :, k * ICOLS:k * ICOLS + w.shape[1]] = w

    return dict(sh=sh, nb=nb, phalf=phalf, kb2=kb2, nch=nch,
                chunk_start=chunk_start, batches=batches, chunk_pos=chunk_pos,
                agg_idx=agg_idx, ind=ind)


def _build_decode_plan(edge_label_index, lsh, sh, nb, phalf):
    """Uniform-across-cores decode plan: labels grouped by (a_half, b_half);
    group sizes padded to the max chunk count over cores."""
    grp_chunks = np.zeros((NC, 4), np.int64)
    per_core = []
    for c in range(NC):
        a = _prow(edge_label_index[0, c * lsh:(c + 1) * lsh].astype(np.int64), sh, nb)
        b_ = _prow(edge_label_index[1, c * lsh:(c + 1) * lsh].astype(np.int64), sh, nb)
        g = (a >= phalf) * 2 + (b_ >= phalf)
        perm = np.argsort(g, kind='stable')
        per_core.append((a[perm], b_[perm], g[perm], perm))
        for gr in range(4):
            n = int((g == gr).sum())
            grp_chunks[c, gr] = (n + P - 1) // P
    kg = grp_chunks.max(axis=0)                  # chunks per group (uniform)
    ldch = int(kg.sum())
    chunk_grp = np.repeat(np.arange(4), kg)
    batches, _ = _batch_chunks(chunk_grp)        # group-pure batches
    g_start = np.zeros(5, np.int64)
    g_start[1:] = np.cumsum(kg)

    slots_a = np.zeros((NC, ldch * P), np.int16)
    slots_b = np.zeros((NC, ldch * P), np.int16)
    slot_lbl = np.full((NC, ldch * P), -1, np.int64)
    for c in range(NC):
        a_s, b_s, g_s, perm = per_core[c]
        for gr in range(4):
            m = np.where(g_s == gr)[0]
            base = g_start[gr] * P
            ah, bh = gr >> 1, gr & 1
            slots_a[c, base:base + len(m)] = (a_s[m] - ah * phalf).astype(np.int16)
            slots_b[c, base:base + len(m)] = (b_s[m] - bh * phalf).astype(np.int16)
            slot_lbl[c, base:base + len(m)] = perm[m]

    idxA = np.zeros((NC, P, len(batches) * ICOLS), np.int16)
    idxB = np.zeros((NC, P, len(batches) * ICOLS), np.int16)
    for c in range(NC):
        for k, (lo, hi, gr) in enumerate(batches):
            wA = _wrap_idx(slots_a[c, lo * P:hi * P])
            wB = _wrap_idx(slots_b[c, lo * P:hi * P])
            idxA[c, :, k * ICOLS:k * ICOLS + wA.shape[1]] = wA
            idxB[c, :, k * ICOLS:k * ICOLS + wB.shape[1]] = wB
    return dict(ldch=ldch, batches=batches, idxA=idxA, idxB=idxB,
                slot_lbl=slot_lbl)


def _build_bass(n_nodes, f_in, meta, decm):
    sh, nb, kb2, nch = meta["sh"], meta["nb"], meta["kb2"], meta["nch"]
    phalf, chunk_start = meta["phalf"], meta["chunk_start"]
    batches, chunk_pos = meta["batches"], meta["chunk_pos"]
    ldch, dbatches = decm["ldch"], decm["batches"]
    f32, bf16, i16 = mybir.dt.float32, mybir.dt.bfloat16, mybir.dt.int16
    KIN = f_in // P
    xdt = bf16 if XBF16 else f32
    wdt = bf16 if WBF16 else f32

    nc = bacc.Bacc(None, target_bir_lowering=False, debug=False, num_devices=NC,
                   num_swdge_queues=2)

    xT = nc.dram_tensor("xT", [KIN, P, sh], xdt, kind="ExternalInput")
    W0 = nc.dram_tensor("W0", [KIN, P, P], xdt, kind="ExternalInput")
    W1 = nc.dram_tensor("W1", [P, P], wdt, kind="ExternalInput")
    W2 = nc.dram_tensor("W2", [P, P], wdt, kind="ExternalInput")
    bcols = nc.dram_tensor("bcols", [P, 3], f32, kind="ExternalInput")
    dinv_blk = nc.dram_tensor("dinv_blk", [P, nb], f32, kind="ExternalInput")
    ident_in = nc.dram_tensor("ident", [P, P], bf16, kind="ExternalInput")
    diag_in = nc.dram_tensor("diag", [P, nb * P], bf16, kind="ExternalInput")
    ind_in = nc.dram_tensor("ind", [P, nch * P], bf16, kind="ExternalInput")
    aggidx_in = nc.dram_tensor("agg_idx", [P, len(batches) * ICOLS], i16,
                               kind="ExternalInput")
    idxA_in = nc.dram_tensor("idxA", [P, len(dbatches) * ICOLS], i16,
                             kind="ExternalInput")
    idxB_in = nc.dram_tensor("idxB", [P, len(dbatches) * ICOLS], i16,
                             kind="ExternalInput")
    logits_out = nc.dram_tensor("logits", [P, ldch], f32, kind="ExternalOutput")

    # permuted-layout shard/full tables: shard row (p*nb + b) = node b*128+p
    shard = [nc.dram_tensor(f"shard{l}", [P, nb, P], bf16) for l in range(4)]
    full = [nc.dram_tensor(f"full{l}", [NC * P * nb, P], bf16,
                           addr_space="Shared") for l in range(4)]

    rg = [list(range(NC))]

    with tile.TileContext(nc) as tc:
        with (
            tc.tile_pool(name="const", bufs=1) as cp,
            tc.tile_pool(name="msg", bufs=8) as mp,
            tc.tile_pool(name="indp", bufs=4) as ip,
            tc.tile_pool(name="dec", bufs=4) as dp,
            tc.tile_pool(name="work", bufs=3) as wp,
            tc.tile_pool(name="pagg", bufs=4, space="PSUM") as pagg,
            tc.tile_pool(name="pwm", bufs=2, space="PSUM") as pwm,
            tc.tile_pool(name="ptr", bufs=2, space="PSUM") as ptr,
        ):
            nc.gpsimd.load_library(library_config.mlp)
            w0 = cp.tile([P, KIN, P], xdt)
            for k in range(KIN):
                nc.sync.dma_start(w0[:, k, :], W0[k, :, :])
            w1 = cp.tile([P, P], wdt)
            nc.sync.dma_start(w1[:], W1[:])
            w2 = cp.tile([P, P], wdt)
            nc.sync.dma_start(w2[:], W2[:])
            bc = cp.tile([P, 3], f32)
            nc.sync.dma_start(bc[:], bcols[:])
            dv = cp.tile([P, nb], f32)
            nc.sync.dma_start(dv[:], dinv_blk[:])
            ident = cp.tile([P, P], bf16)
            nc.sync.dma_start(ident[:], ident_in[:])
            diag = cp.tile([P, nb * P], bf16)
            nc.sync.dma_start(diag[:], diag_in[:])
            shard_sb = cp.tile([P, nb, P], bf16)
            nc.gpsimd.memset(shard_sb[:, nb - 1, :], 0.0)
            aggidx = cp.tile([P, len(batches) * ICOLS], i16)
            nc.sync.dma_start(aggidx[:], aggidx_in[:])
            idxA = cp.tile([P, len(dbatches) * ICOLS], i16)
            nc.sync.dma_start(idxA[:], idxA_in[:])
            idxB = cp.tile([P, len(dbatches) * ICOLS], i16)
            nc.sync.dma_start(idxB[:], idxB_in[:])

            aggT = cp.tile([P, sh], wdt)
            logits_sb = cp.tile([P, ldch], f32)

            def emit_shard_block(psum_h, b, rb):
                """psum [f, rows] -> transpose -> dinv-scale -> shard_sb."""
                t1 = wp.tile([P, P], bf16, tag="t1")
                nc.scalar.activation(t1[:, :rb], psum_h[:, :rb],
                                     mybir.ActivationFunctionType.Copy)
                pt = ptr.tile([P, P], bf16, tag="pt")
                nc.tensor.transpose(pt[:rb, :], t1[:, :rb], ident[:])
                nc.vector.tensor_scalar_mul(shard_sb[:rb, b, :], pt[:rb, :],
                                            dv[:rb, b:b + 1])

            def flush_shard(layer):
                nc.sync.dma_start(shard[layer][:], shard_sb[:])

            # ---- layer 0 table: htil0 = dinv * (x @ W0) ----
            with tc.tile_pool(name="xp", bufs=1) as xp:
                xt = xp.tile([P, KIN, sh], xdt)
                for k in range(KIN):
                    nc.sync.dma_start(xt[:, k, :], xT[k, :, :])
                for wb in range(0, sh, WW):
                    wcols = min(WW, sh - wb)
                    ph = pwm.tile([P, WW], f32, tag="ph")
                    for k in range(KIN):
                        nc.tensor.matmul(ph[:, :wcols], w0[:, k, :],
                                         xt[:, k, wb:wb + wcols],
                                         start=(k == 0), stop=(k == KIN - 1))
                    for j in range(0, wcols, P):
                        rb = min(P, wcols - j)
                        emit_shard_block(ph[:, j:j + rb], (wb + j) // P, rb)
                flush_shard(0)

            def do_allgather(layer):
                nc.gpsimd.collective_compute(
                    "AllGather", mybir.AluOpType.bypass, replica_groups=rg,
                    ins=[shard[layer].ap().opt()], outs=[full[layer].ap().opt()])

            Q38 = bool(int(os.environ.get('GCN_Q38', '1')))

            def gather_batch(pool, tag, table, idx_tile, k, lo, hi, h):
                # part of the batches on SWDGE queue 1: the two queues' DMA
                # rings process transfers in parallel. 3/8 verified correct
                # (1432us); 2/5 corrupts results (timing race).
                n = hi - lo
                if Q38:
                    qn = 1 if k % 8 in (2, 5, 7) else 0
                else:
                    qn = 1 if k % 3 == 2 else 0
                m = pool.tile([P, n, P], bf16, tag=tag)
                nc.gpsimd.dma_gather(
                    out_ap=m[:], in_ap=table[h * phalf:(h + 1) * phalf, :],
                    idxs_ap=idx_tile[:, k * ICOLS:k * ICOLS + n * 8],
                    num_idxs=n * P, num_idxs_reg=n * P, elem_size=P,
                    queue_num=qn)
                return m

            def do_aggregation(layer, relu, w_next=None, next_layer=None):
                tiles = {}

                def chunk_tile(c):
                    k, pos = chunk_pos[c]
                    if k not in tiles:
                        lo, hi, h = batches[k]
                        m = gather_batch(mp, f"m{h}", full[layer],
                                         aggidx, k, lo, hi, h)
                        indb = ip.tile([P, (hi - lo) * P], bf16, tag="ib")
                        nc.sync.dma_start(indb[:], ind_in[:, lo * P:hi * P])
                        tiles[k] = (m, indb, lo)
                    return tiles[k], int(pos)

                for b in range(nb):
                    rb = min(P, sh - b * P)
                    pg = pagg.tile([P, P], f32, tag="pg")
                    ktot = int(kb2[b, 0] + kb2[b, 1])
                    nc.tensor.matmul(pg[:], shard_sb[:, b, :],
                                     diag[:, b * P:(b + 1) * P],
                                     start=True, stop=(ktot == 0))
                    done = 0
                    for h in range(2):
                        for j in range(int(kb2[b, h])):
                            c = int(chunk_start[b, h]) + j
                            (m, indb, blo), pos = chunk_tile(c)
                            done += 1
                            nc.tensor.matmul(pg[:], m[:, pos, :],
                                             indb[:, pos * P:(pos + 1) * P],
                                             start=False, stop=(done == ktot))
                    if relu:
                        nc.scalar.activation(
                            aggT[:, b * P:b * P + rb], pg[:, :rb],
                            mybir.ActivationFunctionType.Relu,
                            bias=bc[:, layer:layer + 1])
                        if w_next is not None and ((b + 1) % 4 == 0 or b == nb - 1):
                            wb0 = (b // 4) * WW
                            wcols = min(WW, sh - wb0)
                            ph = pwm.tile([P, WW], f32, tag="ph")
                            nc.tensor.matmul(ph[:, :wcols], w_next[:],
                                             aggT[:, wb0:wb0 + wcols],
                                             start=True, stop=True)
                            for j in range(0, wcols, P):
                                rb2 = min(P, wcols - j)
                                emit_shard_block(ph[:, j:j + rb2],
                                                 (wb0 + j) // P, rb2)
                    else:
                        # z block: bias, transpose into shard_sb (node-major)
                        zt = wp.tile([P, P], bf16, tag="zt")
                        nc.vector.tensor_scalar_add(zt[:, :rb], pg[:, :rb],
                                                    bc[:, layer:layer + 1])
                        pt = ptr.tile([P, P], bf16, tag="pt")
                        nc.tensor.transpose(pt[:rb, :], zt[:, :rb], ident[:])
                        nc.vector.tensor_copy(shard_sb[:rb, b, :], pt[:rb, :])
                if not relu:
                    flush_shard(3)
                elif w_next is not None:
                    flush_shard(next_layer)

            def do_weight_matmul(w, layer):
                for wb in range(0, sh, WW):
                    wcols = min(WW, sh - wb)
                    ph = pwm.tile([P, WW], f32, tag="ph")
                    nc.tensor.matmul(ph[:, :wcols], w[:], aggT[:, wb:wb + wcols],
                                     start=True, stop=True)
                    for j in range(0, wcols, P):
                        rb = min(P, wcols - j)
                        emit_shard_block(ph[:, j:j + rb], (wb + j) // P, rb)
                flush_shard(layer)

            stage = int(os.environ.get("GCN_STAGE", "5"))
            nc.gpsimd.memset(logits_sb[:], 0.0)
            if stage >= 1:
                do_allgather(0)
            if stage >= 2:
                do_aggregation(0, relu=True, w_next=w1, next_layer=1)
            if stage >= 3:
                do_allgather(1)
                do_aggregation(1, relu=True, w_next=w2, next_layer=2)
            if stage >= 4:
                do_allgather(2)
                do_aggregation(2, relu=False)   # writes z -> shard 3
                do_allgather(3)

            if stage >= 5:
                # ---- decode ----
                for k, (lo, hi, gr) in enumerate(dbatches):
                    w_ = hi - lo
                    za = gather_batch(dp, "za", full[3], idxA, k, lo, hi, gr >> 1)
                    zb = gather_batch(dp, "zb", full[3], idxB, k, lo, hi, gr & 1)
                    prod = dp.tile([P, w_, P], bf16, tag="prod")
                    nc.vector.tensor_tensor(
                        out=prod[:], in0=za[:], in1=zb[:],
                        op=mybir.AluOpType.mult)
                    nc.vector.tensor_reduce(
                        out=logits_sb[:, lo:hi], in_=prod[:],
                        axis=mybir.AxisListType.X, op=mybir.AluOpType.add)
            nc.sync.dma_start(logits_out[:], logits_sb[:])

    nc.compile()
    return nc


def _run(x, edge_index, edge_label_index, W0, b0, W1, b1, W2, b2):
    n, f_in = x.shape
    sh = n // NC
    lsh = edge_label_index.shape[1] // NC
    deg = np.bincount(edge_index[1].astype(np.int64), minlength=n).astype(np.float64) + 1.0
    dinv = (1.0 / np.sqrt(deg)).astype(np.float32)

    meta = _build_plan(n, edge_index, dinv)
    decm = _build_decode_plan(edge_label_index, lsh, sh, meta["nb"], meta["phalf"])
    nc = _build_bass(n, f_in, meta, decm)

    eye = np.eye(P, dtype=ml_dtypes.bfloat16)
    bcol = np.stack([b0, b1, b2], axis=1).astype(np.float32)
    nb = meta["nb"]
    dvb = np.zeros((NC, P, nb), np.float32)
    for c in range(NC):
        d = dinv[c * sh:(c + 1) * sh]
        d = np.pad(d, (0, nb * P - sh))
        dvb[c] = d.reshape(nb, P).T
    KIN = f_in // P
    xdt = ml_dtypes.bfloat16 if XBF16 else np.float32
    wdt = ml_dtypes.bfloat16 if WBF16 else np.float32

    diags = np.zeros((NC, P, nb * P), np.float32)
    for c in range(NC):
        for b in range(nb):
            np.fill_diagonal(diags[c, :, b * P:(b + 1) * P], dvb[c, :, b])
    diags = diags.astype(ml_dtypes.bfloat16)

    in_maps = []
    for c in range(NC):
        xs = x[c * sh:(c + 1) * sh].astype(np.float32)
        xT = np.ascontiguousarray(xs.T.reshape(KIN, P, sh)).astype(xdt)
        in_maps.append({
            "xT": xT,
            "W0": np.ascontiguousarray(W0.reshape(KIN, P, P)).astype(xdt),
            "W1": W1.astype(wdt), "W2": W2.astype(wdt),
            "bcols": bcol, "dinv_blk": dvb[c], "ident": eye,
            "ind": np.ascontiguousarray(meta["ind"][c]),
            "diag": np.ascontiguousarray(diags[c]),
            "agg_idx": np.ascontiguousarray(meta["agg_idx"][c]),
            "idxA": np.ascontiguousarray(decm["idxA"][c]),
            "idxB": np.ascontiguousarray(decm["idxB"][c]),
        })

    res = run_bass_kernel_spmd(nc, in_maps, core_ids=list(range(NC)),
                               trace=bool(os.environ.get("GCN_TRACE")))
    slot_lbl = decm["slot_lbl"]
    logits = np.zeros(NC * lsh, np.float32)
    for c in range(NC):
        lg = np.asarray(res.results[c]["logits"])   # [P, ldch], slot s=ch*P+p
        flat = lg.T.reshape(-1)
        valid = slot_lbl[c] >= 0
        logits[c * lsh + slot_lbl[c][valid]] = flat[valid]
    return logits, res


def kernel(x, edge_index, edge_label_index, W0, b0, W1, b1, W2, b2):
    logits, _ = _run(np.asarray(x), np.asarray(edge_index), np.asarray(edge_label_index),
                     np.asarray(W0), np.asarray(b0), np.asarray(W1), np.asarray(b1),
                     np.asarray(W2), np.asarray(b2))
    return logits


# revision 17
# speedup vs baseline: 1.0881x; 1.0881x over previous
"""GCN edge-prediction kernel for 8 trn2 NeuronCores (Bass/Tile).

Math (per GCNConv layer, PyG semantics with self-loops + symmetric norm):
    h = x @ W;  htil = dinv * h  (row scale)
    out[d] = dinv[d] * sum_{e: s->d, incl self} htil[s] + b
Implemented as:
  - node shard of 6250 rows per core; per-layer bf16 node table AllGathered.
    Tables are stored in PERMUTED row order (node n -> row
    core*6272 + (local%128)*49 + local//128) so the per-layer shard emit is
    ONE contiguous 1.6MB DMA from the node-major SBUF tile instead of ~6k
    256-byte descriptors; gathers just relabel indices host-side.
  - per dst-block (128 nodes) edge chunks of 128 edges, each chunk pure in
    one table half (dma_gather idxs are int16; table > 32k rows); chunk
    order = all lower-half chunks (block-major), then all upper-half chunks,
    so gather batches of up to GB chunks per InstDMAGatherAnt stay
    half-pure; scatter-sum via PE matmul with a one-hot indicator carrying
    dinv[dst]
  - decode: labels permuted into (a_half, b_half) groups; per group, batched
    dma_gather of z rows for both endpoints, one wide mul + 3D-AP reduce per
    batch on DVE; host inverse-permutes the logits
"""
import os
import sys

sys.path.insert(0, "/opt/trn_rl_repo")

import numpy as np
import ml_dtypes

import concourse.bass as bass
import concourse.bacc as bacc
import concourse.mybir as mybir
import concourse.tile as tile
from concourse import library_config
from concourse.bass_utils import run_bass_kernel_spmd

NC = 8
P = 128
GB = int(os.environ.get('GCN_GB', '8'))       # chunks per dma_gather
WW = int(os.environ.get('GCN_WW', '512'))     # weight-matmul moving width
XBF16 = bool(int(os.environ.get('GCN_XBF16', '1')))
WBF16 = bool(int(os.environ.get('GCN_WBF16', '1')))
ICOLS = GB * P // 16                          # idx cols per batch slot


def _prow(n, sh, nb):
    """Node id -> permuted table row (see module docstring)."""
    c, u = n // sh, n % sh
    return c * (nb * P) + (u % P) * nb + u // P


def _wrap_idx(flat):
    """int16 idx list -> wrapped [P, ceil(len/16)] with 8 replicated groups."""
    n = len(flat)
    cols = (n + 15) // 16
    out = np.zeros((P, cols), np.int16)
    j = np.arange(n)
    for g in range(8):
        out[g * 16 + (j % 16), j // 16] = flat
    return out


def _batch_chunks(tags):
    """Greedy batches of <=GB consecutive chunks sharing a tag.

    Returns list of (lo, hi, tag) and per-chunk (batch_idx, pos)."""
    batches = []
    chunk_pos = np.zeros((len(tags), 2), np.int64)
    i = 0
    while i < len(tags):
        t = tags[i]
        j = i
        while j < len(tags) and j - i < GB and tags[j] == t:
            j += 1
        for c in range(i, j):
            chunk_pos[c] = (len(batches), c - i)
        batches.append((i, j, int(t)))
        i = j
    return batches, chunk_pos


def _build_plan(n_nodes, edge_index, dinv):
    """Host-side partitioning: half-pure chunks, A-region then B-region."""
    sh = n_nodes // NC
    nb = (sh + P - 1) // P
    phalf = NC * nb * P // 2
    src, dst = edge_index[0].astype(np.int64), edge_index[1].astype(np.int64)

    core = dst // sh
    blk = (dst % sh) // P
    dl = (dst % sh) % P
    srow = _prow(src, sh, nb)
    half = (srow >= phalf).astype(np.int64)

    counts = np.zeros((NC, nb, 2), np.int64)
    np.add.at(counts, (core, blk, half), 1)
    kb2 = -(-counts.max(axis=0) // P)           # [nb, 2] chunks (can be 0)
    nchA = int(kb2[:, 0].sum())
    nch = nchA + int(kb2[:, 1].sum())
    chunk_start = np.zeros((nb, 2), np.int64)
    chunk_start[:, 0] = np.cumsum(kb2[:, 0]) - kb2[:, 0]
    chunk_start[:, 1] = nchA + np.cumsum(kb2[:, 1]) - kb2[:, 1]
    chunk_half = np.zeros(nch, np.int64)
    chunk_half[nchA:] = 1
    batches, chunk_pos = _batch_chunks(chunk_half)

    idxrel = np.zeros((NC, P, nch), np.int16)
    ind = np.zeros((NC, P, nch * P), np.float32)
    order = np.lexsort((dl, half, blk, core))
    srow_s, blk_s, dl_s, dst_s = srow[order], blk[order], dl[order], dst[order]
    core_s, half_s = core[order], half[order]
    key = (core_s * nb + blk_s) * 2 + half_s
    bounds = np.searchsorted(key, np.arange(NC * nb * 2 + 1) - 0.5)
    for c in range(NC):
        for b in range(nb):
            for h in range(2):
                k = (c * nb + b) * 2 + h
                lo, hi = bounds[k], bounds[k + 1]
                if hi == lo:
                    continue
                slot = np.arange(hi - lo)
                ch = chunk_start[b, h] + slot // P
                lane = slot % P
                idxrel[c, lane, ch] = (srow_s[lo:hi] - h * phalf).astype(np.int16)
                ind[c, lane, ch * P + dl_s[lo:hi]] = dinv[dst_s[lo:hi]]
    ind = ind.astype(ml_dtypes.bfloat16)

    agg_idx = np.zeros((NC, P, len(batches) * ICOLS), np.int16)
    for c in range(NC):
        for k, (lo, hi, h) in enumerate(batches):
            flat = idxrel[c][:, lo:hi].T.reshape(-1)   # chunk-major, lane-minor
            w = _wrap_idx(flat)
            agg_idx[c, :, k * ICOLS:k * ICOLS + w.shape[1]] = w

    return dict(sh=sh, nb=nb, phalf=phalf, kb2=kb2, nch=nch,
                chunk_start=chunk_start, batches=batches, chunk_pos=chunk_pos,
                agg_idx=agg_idx, ind=ind)


def _build_decode_plan(edge_label_index, lsh, sh, nb, phalf):
    """Uniform-across-cores decode plan: labels grouped by (a_half, b_half);
    group sizes padded to the max chunk count over cores."""
    grp_chunks = np.zeros((NC, 4), np.int64)
    per_core = []
    for c in range(NC):
        a = _prow(edge_label_index[0, c * lsh:(c + 1) * lsh].astype(np.int64), sh, nb)
        b_ = _prow(edge_label_index[1, c * lsh:(c + 1) * lsh].astype(np.int64), sh, nb)
        g = (a >= phalf) * 2 + (b_ >= phalf)
        perm = np.argsort(g, kind='stable')
        per_core.append((a[perm], b_[perm], g[perm], perm))
        for gr in range(4):
            n = int((g == gr).sum())
            grp_chunks[c, gr] = (n + P - 1) // P
    kg = grp_chunks.max(axis=0)                  # chunks per group (uniform)
    ldch = int(kg.sum())
    chunk_grp = np.repeat(np.arange(4), kg)
    batches, _ = _batch_chunks(chunk_grp)        # group-pure batches
    g_start = np.zeros(5, np.int64)
    g_start[1:] = np.cumsum(kg)

    slots_a = np.zeros((NC, ldch * P), np.int16)
    slots_b = np.zeros((NC, ldch * P), np.int16)
    slot_lbl = np.full((NC, ldch * P), -1, np.int64)
    for c in range(NC):
        a_s, b_s, g_s, perm = per_core[c]
        for gr in range(4):
            m = np.where(g_s == gr)[0]
            base = g_start[gr] * P
            ah, bh = gr >> 1, gr & 1
            slots_a[c, base:base + len(m)] = (a_s[m] - ah * phalf).astype(np.int16)
            slots_b[c, base:base + len(m)] = (b_s[m] - bh * phalf).astype(np.int16)
            slot_lbl[c, base:base + len(m)] = perm[m]

    idxA = np.zeros((NC, P, len(batches) * ICOLS), np.int16)
    idxB = np.zeros((NC, P, len(batches) * ICOLS), np.int16)
    for c in range(NC):
        for k, (lo, hi, gr) in enumerate(batches):
            wA = _wrap_idx(slots_a[c, lo * P:hi * P])
            wB = _wrap_idx(slots_b[c, lo * P:hi * P])
            idxA[c, :, k * ICOLS:k * ICOLS + wA.shape[1]] = wA
            idxB[c, :, k * ICOLS:k * ICOLS + wB.shape[1]] = wB
    return dict(ldch=ldch, batches=batches, idxA=idxA, idxB=idxB,
                slot_lbl=slot_lbl)


def _build_bass(n_nodes, f_in, meta, decm):
    sh, nb, kb2, nch = meta["sh"], meta["nb"], meta["kb2"], meta["nch"]
    phalf, chunk_start = meta["phalf"], meta["chunk_start"]
    batches, chunk_pos = meta["batches"], meta["chunk_pos"]
    ldch, dbatches = decm["ldch"], decm["batches"]
    f32, bf16, i16 = mybir.dt.float32, mybir.dt.bfloat16, mybir.dt.int16
    KIN = f_in // P
    xdt = bf16 if XBF16 else f32
    wdt = bf16 if WBF16 else f32

    q3 = bool(int(os.environ.get('GCN_Q3', '0')))
    nc = bacc.Bacc(None, target_bir_lowering=False, debug=False, num_devices=NC,
                   num_swdge_queues=3 if q3 else 2)

    xT = nc.dram_tensor("xT", [KIN, P, sh], xdt, kind="ExternalInput")
    W0 = nc.dram_tensor("W0", [KIN, P, P], xdt, kind="ExternalInput")
    W1 = nc.dram_tensor("W1", [P, P], wdt, kind="ExternalInput")
    W2 = nc.dram_tensor("W2", [P, P], wdt, kind="ExternalInput")
    bcols = nc.dram_tensor("bcols", [P, 3], f32, kind="ExternalInput")
    dinv_blk = nc.dram_tensor("dinv_blk", [P, nb], f32, kind="ExternalInput")
    ident_in = nc.dram_tensor("ident", [P, P], bf16, kind="ExternalInput")
    diag_in = nc.dram_tensor("diag", [P, nb * P], bf16, kind="ExternalInput")
    ind_in = nc.dram_tensor("ind", [P, nch * P], bf16, kind="ExternalInput")
    aggidx_in = nc.dram_tensor("agg_idx", [P, len(batches) * ICOLS], i16,
                               kind="ExternalInput")
    idxA_in = nc.dram_tensor("idxA", [P, len(dbatches) * ICOLS], i16,
                             kind="ExternalInput")
    idxB_in = nc.dram_tensor("idxB", [P, len(dbatches) * ICOLS], i16,
                             kind="ExternalInput")
    logits_out = nc.dram_tensor("logits", [P, ldch], f32, kind="ExternalOutput")

    # permuted-layout shard/full tables: shard row (p*nb + b) = node b*128+p
    shard = [nc.dram_tensor(f"shard{l}", [P, nb, P], bf16) for l in range(4)]
    full = [nc.dram_tensor(f"full{l}", [NC * P * nb, P], bf16,
                           addr_space="Shared") for l in range(4)]

    rg = [list(range(NC))]

    with tile.TileContext(nc) as tc:
        with (
            tc.tile_pool(name="const", bufs=1) as cp,
            tc.tile_pool(name="msg", bufs=8) as mp,
            tc.tile_pool(name="indp", bufs=4) as ip,
            tc.tile_pool(name="dec", bufs=4) as dp,
            tc.tile_pool(name="work", bufs=3) as wp,
            tc.tile_pool(name="pagg", bufs=4, space="PSUM") as pagg,
            tc.tile_pool(name="pwm", bufs=2, space="PSUM") as pwm,
            tc.tile_pool(name="ptr", bufs=2, space="PSUM") as ptr,
        ):
            nc.gpsimd.load_library(library_config.mlp)
            w0 = cp.tile([P, KIN, P], xdt)
            for k in range(KIN):
                nc.sync.dma_start(w0[:, k, :], W0[k, :, :])
            w1 = cp.tile([P, P], wdt)
            nc.sync.dma_start(w1[:], W1[:])
            w2 = cp.tile([P, P], wdt)
            nc.sync.dma_start(w2[:], W2[:])
            bc = cp.tile([P, 3], f32)
            nc.sync.dma_start(bc[:], bcols[:])
            dv = cp.tile([P, nb], f32)
            nc.sync.dma_start(dv[:], dinv_blk[:])
            ident = cp.tile([P, P], bf16)
            nc.sync.dma_start(ident[:], ident_in[:])
            diag = cp.tile([P, nb * P], bf16)
            nc.sync.dma_start(diag[:], diag_in[:])
            shard_sb = cp.tile([P, nb, P], bf16)
            nc.gpsimd.memset(shard_sb[:, nb - 1, :], 0.0)
            aggidx = cp.tile([P, len(batches) * ICOLS], i16)
            nc.sync.dma_start(aggidx[:], aggidx_in[:])
            idxA = cp.tile([P, len(dbatches) * ICOLS], i16)
            nc.sync.dma_start(idxA[:], idxA_in[:])
            idxB = cp.tile([P, len(dbatches) * ICOLS], i16)
            nc.sync.dma_start(idxB[:], idxB_in[:])

            aggT = cp.tile([P, sh], wdt)
            logits_sb = cp.tile([P, ldch], f32)

            def emit_shard_block(psum_h, b, rb):
                """psum [f, rows] -> transpose -> dinv-scale -> shard_sb."""
                t1 = wp.tile([P, P], bf16, tag="t1")
                nc.scalar.activation(t1[:, :rb], psum_h[:, :rb],
                                     mybir.ActivationFunctionType.Copy)
                pt = ptr.tile([P, P], bf16, tag="pt")
                nc.tensor.transpose(pt[:rb, :], t1[:, :rb], ident[:])
                nc.vector.tensor_scalar_mul(shard_sb[:rb, b, :], pt[:rb, :],
                                            dv[:rb, b:b + 1])

            def flush_shard(layer):
                nc.sync.dma_start(shard[layer][:], shard_sb[:])

            # ---- layer 0 table: htil0 = dinv * (x @ W0) ----
            with tc.tile_pool(name="xp", bufs=1) as xp:
                xt = xp.tile([P, KIN, sh], xdt)
                for k in range(KIN):
                    nc.sync.dma_start(xt[:, k, :], xT[k, :, :])
                for wb in range(0, sh, WW):
                    wcols = min(WW, sh - wb)
                    ph = pwm.tile([P, WW], f32, tag="ph")
                    for k in range(KIN):
                        nc.tensor.matmul(ph[:, :wcols], w0[:, k, :],
                                         xt[:, k, wb:wb + wcols],
                                         start=(k == 0), stop=(k == KIN - 1))
                    for j in range(0, wcols, P):
                        rb = min(P, wcols - j)
                        emit_shard_block(ph[:, j:j + rb], (wb + j) // P, rb)
                flush_shard(0)

            def do_allgather(layer):
                nc.gpsimd.collective_compute(
                    "AllGather", mybir.AluOpType.bypass, replica_groups=rg,
                    ins=[shard[layer].ap().opt()], outs=[full[layer].ap().opt()])

            Q38 = bool(int(os.environ.get('GCN_Q38', '1')))
            Q3 = bool(int(os.environ.get('GCN_Q3', '0')))

            def gather_batch(pool, tag, table, idx_tile, k, lo, hi, h):
                # part of the batches on SWDGE queue 1: the two queues' DMA
                # rings process transfers in parallel. 3/8 verified correct
                # (1432us); 2/5 corrupts results (timing race).
                n = hi - lo
                if Q3:
                    qn = (1 if k % 8 in (2, 5, 7) else
                          2 if k % 8 == 4 else 0)
                elif Q38:
                    qn = 1 if k % 8 in (2, 5, 7) else 0
                else:
                    qn = 1 if k % 3 == 2 else 0
                m = pool.tile([P, n, P], bf16, tag=tag)
                nc.gpsimd.dma_gather(
                    out_ap=m[:], in_ap=table[h * phalf:(h + 1) * phalf, :],
                    idxs_ap=idx_tile[:, k * ICOLS:k * ICOLS + n * 8],
                    num_idxs=n * P, num_idxs_reg=n * P, elem_size=P,
                    queue_num=qn)
                return m

            def do_aggregation(layer, relu, w_next=None, next_layer=None):
                tiles = {}

                def chunk_tile(c):
                    k, pos = chunk_pos[c]
                    if k not in tiles:
                        lo, hi, h = batches[k]
                        m = gather_batch(mp, f"m{h}", full[layer],
                                         aggidx, k, lo, hi, h)
                        indb = ip.tile([P, (hi - lo) * P], bf16, tag="ib")
                        nc.sync.dma_start(indb[:], ind_in[:, lo * P:hi * P])
                        tiles[k] = (m, indb, lo)
                    return tiles[k], int(pos)

                for b in range(nb):
                    rb = min(P, sh - b * P)
                    pg = pagg.tile([P, P], f32, tag="pg")
                    ktot = int(kb2[b, 0] + kb2[b, 1])
                    nc.tensor.matmul(pg[:], shard_sb[:, b, :],
                                     diag[:, b * P:(b + 1) * P],
                                     start=True, stop=(ktot == 0))
                    done = 0
                    for h in range(2):
                        for j in range(int(kb2[b, h])):
                            c = int(chunk_start[b, h]) + j
                            (m, indb, blo), pos = chunk_tile(c)
                            done += 1
                            nc.tensor.matmul(pg[:], m[:, pos, :],
                                             indb[:, pos * P:(pos + 1) * P],
                                             start=False, stop=(done == ktot))
                    if relu:
                        nc.scalar.activation(
                            aggT[:, b * P:b * P + rb], pg[:, :rb],
                            mybir.ActivationFunctionType.Relu,
                            bias=bc[:, layer:layer + 1])
                        if w_next is not None and ((b + 1) % 4 == 0 or b == nb - 1):
                            wb0 = (b // 4) * WW
                            wcols = min(WW, sh - wb0)
                            ph = pwm.tile([P, WW], f32, tag="ph")
                            nc.tensor.matmul(ph[:, :wcols], w_next[:],
                                             aggT[:, wb0:wb0 + wcols],
                                             start=True, stop=True)
                            for j in range(0, wcols, P):
                                rb2 = min(P, wcols - j)
                                emit_shard_block(ph[:, j:j + rb2],
                                                 (wb0 + j) // P, rb2)
                    else:
                        # z block: bias, transpose into shard_sb (node-major)
                        zt = wp.tile([P, P], bf16, tag="zt")
                        nc.vector.tensor_scalar_add(zt[:, :rb], pg[:, :rb],
                                                    bc[:, layer:layer + 1])
                        pt = ptr.tile([P, P], bf16, tag="pt")
                        nc.tensor.transpose(pt[:rb, :], zt[:, :rb], ident[:])
                        nc.vector.tensor_copy(shard_sb[:rb, b, :], pt[:rb, :])
                if not relu:
                    flush_shard(3)
                elif w_next is not None:
                    flush_shard(next_layer)

            def do_weight_matmul(w, layer):
                for wb in range(0, sh, WW):
                    wcols = min(WW, sh - wb)
                    ph = pwm.tile([P, WW], f32, tag="ph")
                    nc.tensor.matmul(ph[:, :wcols], w[:], aggT[:, wb:wb + wcols],
                                     start=True, stop=True)
                    for j in range(0, wcols, P):
                        rb = min(P, wcols - j)
                        emit_shard_block(ph[:, j:j + rb], (wb + j) // P, rb)
                flush_shard(layer)

            stage = int(os.environ.get("GCN_STAGE", "5"))
            nc.gpsimd.memset(logits_sb[:], 0.0)
            if stage >= 1:
                do_allgather(0)
            if stage >= 2:
                do_aggregation(0, relu=True, w_next=w1, next_layer=1)
            if stage >= 3:
                do_allgather(1)
                do_aggregation(1, relu=True, w_next=w2, next_layer=2)
            if stage >= 4:
                do_allgather(2)
                do_aggregation(2, relu=False)   # writes z -> shard 3
                do_allgather(3)

            if stage >= 5:
                # ---- decode ----
                for k, (lo, hi, gr) in enumerate(dbatches):
                    w_ = hi - lo
                    za = gather_batch(dp, "za", full[3], idxA, k, lo, hi, gr >> 1)
                    zb = gather_batch(dp, "zb", full[3], idxB, k, lo, hi, gr & 1)
                    prod = dp.tile([P, w_, P], bf16, tag="prod")
                    nc.vector.tensor_tensor(
                        out=prod[:], in0=za[:], in1=zb[:],
                        op=mybir.AluOpType.mult)
                    nc.vector.tensor_reduce(
                        out=logits_sb[:, lo:hi], in_=prod[:],
                        axis=mybir.AxisListType.X, op=mybir.AluOpType.add)
            nc.sync.dma_start(logits_out[:], logits_sb[:])

    nc.compile()
    return nc


def _run(x, edge_index, edge_label_index, W0, b0, W1, b1, W2, b2):
    n, f_in = x.shape
    sh = n // NC
    lsh = edge_label_index.shape[1] // NC
    deg = np.bincount(edge_index[1].astype(np.int64), minlength=n).astype(np.float64) + 1.0
    dinv = (1.0 / np.sqrt(deg)).astype(np.float32)

    meta = _build_plan(n, edge_index, dinv)
    decm = _build_decode_plan(edge_label_index, lsh, sh, meta["nb"], meta["phalf"])
    nc = _build_bass(n, f_in, meta, decm)

    eye = np.eye(P, dtype=ml_dtypes.bfloat16)
    bcol = np.stack([b0, b1, b2], axis=1).astype(np.float32)
    nb = meta["nb"]
    dvb = np.zeros((NC, P, nb), np.float32)
    for c in range(NC):
        d = dinv[c * sh:(c + 1) * sh]
        d = np.pad(d, (0, nb * P - sh))
        dvb[c] = d.reshape(nb, P).T
    KIN = f_in // P
    xdt = ml_dtypes.bfloat16 if XBF16 else np.float32
    wdt = ml_dtypes.bfloat16 if WBF16 else np.float32

    diags = np.zeros((NC, P, nb * P), np.float32)
    for c in range(NC):
        for b in range(nb):
            np.fill_diagonal(diags[c, :, b * P:(b + 1) * P], dvb[c, :, b])
    diags = diags.astype(ml_dtypes.bfloat16)

    in_maps = []
    for c in range(NC):
        xs = x[c * sh:(c + 1) * sh].astype(np.float32)
        xT = np.ascontiguousarray(xs.T.reshape(KIN, P, sh)).astype(xdt)
        in_maps.append({
            "xT": xT,
            "W0": np.ascontiguousarray(W0.reshape(KIN, P, P)).astype(xdt),
            "W1": W1.astype(wdt), "W2": W2.astype(wdt),
            "bcols": bcol, "dinv_blk": dvb[c], "ident": eye,
            "ind": np.ascontiguousarray(meta["ind"][c]),
            "diag": np.ascontiguousarray(diags[c]),
            "agg_idx": np.ascontiguousarray(meta["agg_idx"][c]),
            "idxA": np.ascontiguousarray(decm["idxA"][c]),
            "idxB": np.ascontiguousarray(decm["idxB"][c]),
        })

    res = run_bass_kernel_spmd(nc, in_maps, core_ids=list(range(NC)),
                               trace=bool(os.environ.get("GCN_TRACE")))
    slot_lbl = decm["slot_lbl"]
    logits = np.zeros(NC * lsh, np.float32)
    for c in range(NC):
        lg = np.asarray(res.results[c]["logits"])   # [P, ldch], slot s=ch*P+p
        flat = lg.T.reshape(-1)
        valid = slot_lbl[c] >= 0
        logits[c * lsh + slot_lbl[c][valid]] = flat[valid]
    return logits, res


def kernel(x, edge_index, edge_label_index, W0, b0, W1, b1, W2, b2):
    logits, _ = _run(np.asarray(x), np.asarray(edge_index), np.asarray(edge_label_index),
                     np.asarray(W0), np.asarray(b0), np.asarray(W1), np.asarray(b1),
                     np.asarray(W2), np.asarray(b2))
    return logits


# revision 19
# speedup vs baseline: 1.1452x; 1.0525x over previous
"""GCN edge-prediction kernel for 8 trn2 NeuronCores (Bass/Tile).

Math (per GCNConv layer, PyG semantics with self-loops + symmetric norm):
    h = x @ W;  htil = dinv * h  (row scale)
    out[d] = dinv[d] * sum_{e: s->d, incl self} htil[s] + b
Implemented as:
  - node shard of 6250 rows per core; per-layer bf16 node table AllGathered.
    Tables are stored in PERMUTED row order (node n -> row
    core*6272 + (local%128)*49 + local//128) so the per-layer shard emit is
    ONE contiguous 1.6MB DMA from the node-major SBUF tile instead of ~6k
    256-byte descriptors; gathers just relabel indices host-side.
  - per dst-block (128 nodes) edge chunks of 128 edges, each chunk pure in
    one table half (dma_gather idxs are int16; table > 32k rows); chunk
    order = all lower-half chunks (block-major), then all upper-half chunks,
    so gather batches of up to GB chunks per InstDMAGatherAnt stay
    half-pure; scatter-sum via PE matmul with a one-hot indicator carrying
    dinv[dst]
  - decode: labels permuted into (a_half, b_half) groups; per group, batched
    dma_gather of z rows for both endpoints, one wide mul + 3D-AP reduce per
    batch on DVE; host inverse-permutes the logits
"""
import os
import sys

sys.path.insert(0, "/opt/trn_rl_repo")

import numpy as np
import ml_dtypes

import concourse.bass as bass
import concourse.bacc as bacc
import concourse.mybir as mybir
import concourse.tile as tile
from concourse import library_config
from concourse.bass_utils import run_bass_kernel_spmd

NC = 8
P = 128
GB = int(os.environ.get('GCN_GB', '8'))       # chunks per dma_gather
WW = int(os.environ.get('GCN_WW', '512'))     # weight-matmul moving width
XBF16 = bool(int(os.environ.get('GCN_XBF16', '1')))
WBF16 = bool(int(os.environ.get('GCN_WBF16', '1')))
ICOLS = GB * P // 16                          # idx cols per batch slot


def _prow(n, sh, nb):
    """Node id -> permuted table row (see module docstring)."""
    c, u = n // sh, n % sh
    return c * (nb * P) + (u % P) * nb + u // P


def _wrap_idx(flat):
    """int16 idx list -> wrapped [P, ceil(len/16)] with 8 replicated groups."""
    n = len(flat)
    cols = (n + 15) // 16
    out = np.zeros((P, cols), np.int16)
    j = np.arange(n)
    for g in range(8):
        out[g * 16 + (j % 16), j // 16] = flat
    return out


def _batch_chunks(tags):
    """Greedy batches of <=GB consecutive chunks sharing a tag.

    Returns list of (lo, hi, tag) and per-chunk (batch_idx, pos)."""
    batches = []
    chunk_pos = np.zeros((len(tags), 2), np.int64)
    i = 0
    while i < len(tags):
        t = tags[i]
        j = i
        while j < len(tags) and j - i < GB and tags[j] == t:
            j += 1
        for c in range(i, j):
            chunk_pos[c] = (len(batches), c - i)
        batches.append((i, j, int(t)))
        i = j
    return batches, chunk_pos


def _build_plan(n_nodes, edge_index, dinv):
    """Host-side partitioning: half-pure chunks, A-region then B-region."""
    sh = n_nodes // NC
    nb = (sh + P - 1) // P
    phalf = NC * nb * P // 2
    src, dst = edge_index[0].astype(np.int64), edge_index[1].astype(np.int64)

    core = dst // sh
    blk = (dst % sh) // P
    dl = (dst % sh) % P
    srow = _prow(src, sh, nb)
    half = (srow >= phalf).astype(np.int64)

    counts = np.zeros((NC, nb, 2), np.int64)
    np.add.at(counts, (core, blk, half), 1)
    kb2 = -(-counts.max(axis=0) // P)           # [nb, 2] chunks (can be 0)
    nchA = int(kb2[:, 0].sum())
    nch = nchA + int(kb2[:, 1].sum())
    chunk_start = np.zeros((nb, 2), np.int64)
    chunk_start[:, 0] = np.cumsum(kb2[:, 0]) - kb2[:, 0]
    chunk_start[:, 1] = nchA + np.cumsum(kb2[:, 1]) - kb2[:, 1]
    chunk_half = np.zeros(nch, np.int64)
    chunk_half[nchA:] = 1
    batches, chunk_pos = _batch_chunks(chunk_half)

    idxrel = np.zeros((NC, P, nch), np.int16)
    ind = np.zeros((NC, P, nch * P), np.float32)
    order = np.lexsort((dl, half, blk, core))
    srow_s, blk_s, dl_s, dst_s = srow[order], blk[order], dl[order], dst[order]
    core_s, half_s = core[order], half[order]
    key = (core_s * nb + blk_s) * 2 + half_s
    bounds = np.searchsorted(key, np.arange(NC * nb * 2 + 1) - 0.5)
    for c in range(NC):
        for b in range(nb):
            for h in range(2):
                k = (c * nb + b) * 2 + h
                lo, hi = bounds[k], bounds[k + 1]
                if hi == lo:
                    continue
                slot = np.arange(hi - lo)
                ch = chunk_start[b, h] + slot // P
                lane = slot % P
                idxrel[c, lane, ch] = (srow_s[lo:hi] - h * phalf).astype(np.int16)
                ind[c, lane, ch * P + dl_s[lo:hi]] = dinv[dst_s[lo:hi]]
    ind = ind.astype(ml_dtypes.bfloat16)

    agg_idx = np.zeros((NC, P, len(batches) * ICOLS), np.int16)
    for c in range(NC):
        for k, (lo, hi, h) in enumerate(batches):
            flat = idxrel[c][:, lo:hi].T.reshape(-1)   # chunk-major, lane-minor
            w = _wrap_idx(flat)
            agg_idx[c, :, k * ICOLS:k * ICOLS + w.shape[1]] = w

    return dict(sh=sh, nb=nb, phalf=phalf, kb2=kb2, nch=nch,
                chunk_start=chunk_start, batches=batches, chunk_pos=chunk_pos,
                agg_idx=agg_idx, ind=ind)


def _build_decode_plan(edge_label_index, lsh, sh, nb, phalf):
    """Uniform-across-cores decode plan: labels grouped by (a_half, b_half);
    group sizes padded to the max chunk count over cores."""
    grp_chunks = np.zeros((NC, 4), np.int64)
    per_core = []
    for c in range(NC):
        a = _prow(edge_label_index[0, c * lsh:(c + 1) * lsh].astype(np.int64), sh, nb)
        b_ = _prow(edge_label_index[1, c * lsh:(c + 1) * lsh].astype(np.int64), sh, nb)
        g = (a >= phalf) * 2 + (b_ >= phalf)
        perm = np.argsort(g, kind='stable')
        per_core.append((a[perm], b_[perm], g[perm], perm))
        for gr in range(4):
            n = int((g == gr).sum())
            grp_chunks[c, gr] = (n + P - 1) // P
    kg = grp_chunks.max(axis=0)                  # chunks per group (uniform)
    ldch = int(kg.sum())
    chunk_grp = np.repeat(np.arange(4), kg)
    batches, _ = _batch_chunks(chunk_grp)        # group-pure batches
    g_start = np.zeros(5, np.int64)
    g_start[1:] = np.cumsum(kg)

    slots_a = np.zeros((NC, ldch * P), np.int16)
    slots_b = np.zeros((NC, ldch * P), np.int16)
    slot_lbl = np.full((NC, ldch * P), -1, np.int64)
    for c in range(NC):
        a_s, b_s, g_s, perm = per_core[c]
        for gr in range(4):
            m = np.where(g_s == gr)[0]
            base = g_start[gr] * P
            ah, bh = gr >> 1, gr & 1
            slots_a[c, base:base + len(m)] = (a_s[m] - ah * phalf).astype(np.int16)
            slots_b[c, base:base + len(m)] = (b_s[m] - bh * phalf).astype(np.int16)
            slot_lbl[c, base:base + len(m)] = perm[m]

    idxA = np.zeros((NC, P, len(batches) * ICOLS), np.int16)
    idxB = np.zeros((NC, P, len(batches) * ICOLS), np.int16)
    for c in range(NC):
        for k, (lo, hi, gr) in enumerate(batches):
            wA = _wrap_idx(slots_a[c, lo * P:hi * P])
            wB = _wrap_idx(slots_b[c, lo * P:hi * P])
            idxA[c, :, k * ICOLS:k * ICOLS + wA.shape[1]] = wA
            idxB[c, :, k * ICOLS:k * ICOLS + wB.shape[1]] = wB
    return dict(ldch=ldch, batches=batches, idxA=idxA, idxB=idxB,
                slot_lbl=slot_lbl)


def _build_bass(n_nodes, f_in, meta, decm):
    sh, nb, kb2, nch = meta["sh"], meta["nb"], meta["kb2"], meta["nch"]
    phalf, chunk_start = meta["phalf"], meta["chunk_start"]
    batches, chunk_pos = meta["batches"], meta["chunk_pos"]
    ldch, dbatches = decm["ldch"], decm["batches"]
    f32, bf16, i16 = mybir.dt.float32, mybir.dt.bfloat16, mybir.dt.int16
    KIN = f_in // P
    xdt = bf16 if XBF16 else f32
    wdt = bf16 if WBF16 else f32

    q3 = bool(int(os.environ.get('GCN_Q3', '1')))
    q4 = bool(int(os.environ.get('GCN_Q4', '0')))
    nc = bacc.Bacc(None, target_bir_lowering=False, debug=False, num_devices=NC,
                   num_swdge_queues=4 if q4 else (3 if q3 else 2))

    xT = nc.dram_tensor("xT", [KIN, P, sh], xdt, kind="ExternalInput")
    W0 = nc.dram_tensor("W0", [KIN, P, P], xdt, kind="ExternalInput")
    W1 = nc.dram_tensor("W1", [P, P], wdt, kind="ExternalInput")
    W2 = nc.dram_tensor("W2", [P, P], wdt, kind="ExternalInput")
    bcols = nc.dram_tensor("bcols", [P, 3], f32, kind="ExternalInput")
    dinv_blk = nc.dram_tensor("dinv_blk", [P, nb], f32, kind="ExternalInput")
    ident_in = nc.dram_tensor("ident", [P, P], bf16, kind="ExternalInput")
    diag_in = nc.dram_tensor("diag", [P, nb * P], bf16, kind="ExternalInput")
    ind_in = nc.dram_tensor("ind", [P, nch * P], bf16, kind="ExternalInput")
    aggidx_in = nc.dram_tensor("agg_idx", [P, len(batches) * ICOLS], i16,
                               kind="ExternalInput")
    idxA_in = nc.dram_tensor("idxA", [P, len(dbatches) * ICOLS], i16,
                             kind="ExternalInput")
    idxB_in = nc.dram_tensor("idxB", [P, len(dbatches) * ICOLS], i16,
                             kind="ExternalInput")
    logits_out = nc.dram_tensor("logits", [P, ldch], f32, kind="ExternalOutput")

    # permuted-layout shard/full tables: shard row (p*nb + b) = node b*128+p
    shard = [nc.dram_tensor(f"shard{l}", [P, nb, P], bf16) for l in range(4)]
    full = [nc.dram_tensor(f"full{l}", [NC * P * nb, P], bf16,
                           addr_space="Shared") for l in range(4)]

    rg = [list(range(NC))]

    with tile.TileContext(nc) as tc:
        with (
            tc.tile_pool(name="const", bufs=1) as cp,
            tc.tile_pool(name="msg", bufs=8) as mp,
            tc.tile_pool(name="indp", bufs=4) as ip,
            tc.tile_pool(name="dec", bufs=4) as dp,
            tc.tile_pool(name="work", bufs=3) as wp,
            tc.tile_pool(name="pagg", bufs=4, space="PSUM") as pagg,
            tc.tile_pool(name="pwm", bufs=2, space="PSUM") as pwm,
            tc.tile_pool(name="ptr", bufs=2, space="PSUM") as ptr,
        ):
            nc.gpsimd.load_library(library_config.mlp)
            w0 = cp.tile([P, KIN, P], xdt)
            for k in range(KIN):
                nc.sync.dma_start(w0[:, k, :], W0[k, :, :])
            w1 = cp.tile([P, P], wdt)
            nc.sync.dma_start(w1[:], W1[:])
            w2 = cp.tile([P, P], wdt)
            nc.sync.dma_start(w2[:], W2[:])
            bc = cp.tile([P, 3], f32)
            nc.sync.dma_start(bc[:], bcols[:])
            dv = cp.tile([P, nb], f32)
            nc.sync.dma_start(dv[:], dinv_blk[:])
            ident = cp.tile([P, P], bf16)
            nc.sync.dma_start(ident[:], ident_in[:])
            diag = cp.tile([P, nb * P], bf16)
            nc.sync.dma_start(diag[:], diag_in[:])
            shard_sb = cp.tile([P, nb, P], bf16)
            nc.gpsimd.memset(shard_sb[:, nb - 1, :], 0.0)
            aggidx = cp.tile([P, len(batches) * ICOLS], i16)
            nc.sync.dma_start(aggidx[:], aggidx_in[:])
            idxA = cp.tile([P, len(dbatches) * ICOLS], i16)
            nc.sync.dma_start(idxA[:], idxA_in[:])
            idxB = cp.tile([P, len(dbatches) * ICOLS], i16)
            nc.sync.dma_start(idxB[:], idxB_in[:])

            aggT = cp.tile([P, sh], wdt)
            logits_sb = cp.tile([P, ldch], f32)

            def emit_shard_block(psum_h, b, rb):
                """psum [f, rows] -> transpose -> dinv-scale -> shard_sb."""
                t1 = wp.tile([P, P], bf16, tag="t1")
                nc.scalar.activation(t1[:, :rb], psum_h[:, :rb],
                                     mybir.ActivationFunctionType.Copy)
                pt = ptr.tile([P, P], bf16, tag="pt")
                nc.tensor.transpose(pt[:rb, :], t1[:, :rb], ident[:])
                nc.vector.tensor_scalar_mul(shard_sb[:rb, b, :], pt[:rb, :],
                                            dv[:rb, b:b + 1])

            def flush_shard(layer):
                nc.sync.dma_start(shard[layer][:], shard_sb[:])

            # ---- layer 0 table: htil0 = dinv * (x @ W0) ----
            with tc.tile_pool(name="xp", bufs=1) as xp:
                xt = xp.tile([P, KIN, sh], xdt)
                for k in range(KIN):
                    nc.sync.dma_start(xt[:, k, :], xT[k, :, :])
                for wb in range(0, sh, WW):
                    wcols = min(WW, sh - wb)
                    ph = pwm.tile([P, WW], f32, tag="ph")
                    for k in range(KIN):
                        nc.tensor.matmul(ph[:, :wcols], w0[:, k, :],
                                         xt[:, k, wb:wb + wcols],
                                         start=(k == 0), stop=(k == KIN - 1))
                    for j in range(0, wcols, P):
                        rb = min(P, wcols - j)
                        emit_shard_block(ph[:, j:j + rb], (wb + j) // P, rb)
                flush_shard(0)

            def do_allgather(layer):
                nc.gpsimd.collective_compute(
                    "AllGather", mybir.AluOpType.bypass, replica_groups=rg,
                    ins=[shard[layer].ap().opt()], outs=[full[layer].ap().opt()])

            Q38 = bool(int(os.environ.get('GCN_Q38', '1')))
            Q3 = bool(int(os.environ.get('GCN_Q3', '1')))
            Q4 = bool(int(os.environ.get('GCN_Q4', '0')))

            def gather_batch(pool, tag, table, idx_tile, k, lo, hi, h):
                # part of the batches on SWDGE queue 1: the two queues' DMA
                # rings process transfers in parallel. 3/8 verified correct
                # (1432us); 2/5 corrupts results (timing race).
                n = hi - lo
                if Q4:
                    qn = (1 if k % 8 in (2, 5, 7) else
                          2 if k % 8 == 4 else
                          3 if k % 8 == 1 else 0)
                elif Q3:
                    qn = (1 if k % 8 in (2, 5, 7) else
                          2 if k % 8 == 4 else 0)
                elif Q38:
                    qn = 1 if k % 8 in (2, 5, 7) else 0
                else:
                    qn = 1 if k % 3 == 2 else 0
                m = pool.tile([P, n, P], bf16, tag=tag)
                nc.gpsimd.dma_gather(
                    out_ap=m[:], in_ap=table[h * phalf:(h + 1) * phalf, :],
                    idxs_ap=idx_tile[:, k * ICOLS:k * ICOLS + n * 8],
                    num_idxs=n * P, num_idxs_reg=n * P, elem_size=P,
                    queue_num=qn)
                return m

            def do_aggregation(layer, relu, w_next=None, next_layer=None):
                tiles = {}

                def chunk_tile(c):
                    k, pos = chunk_pos[c]
                    if k not in tiles:
                        lo, hi, h = batches[k]
                        m = gather_batch(mp, f"m{h}", full[layer],
                                         aggidx, k, lo, hi, h)
                        indb = ip.tile([P, (hi - lo) * P], bf16, tag="ib")
                        nc.sync.dma_start(indb[:], ind_in[:, lo * P:hi * P])
                        tiles[k] = (m, indb, lo)
                    return tiles[k], int(pos)

                for b in range(nb):
                    rb = min(P, sh - b * P)
                    pg = pagg.tile([P, P], f32, tag="pg")
                    ktot = int(kb2[b, 0] + kb2[b, 1])
                    nc.tensor.matmul(pg[:], shard_sb[:, b, :],
                                     diag[:, b * P:(b + 1) * P],
                                     start=True, stop=(ktot == 0))
                    done = 0
                    for h in range(2):
                        for j in range(int(kb2[b, h])):
                            c = int(chunk_start[b, h]) + j
                            (m, indb, blo), pos = chunk_tile(c)
                            done += 1
                            nc.tensor.matmul(pg[:], m[:, pos, :],
                                             indb[:, pos * P:(pos + 1) * P],
                                             start=False, stop=(done == ktot))
                    if relu:
                        nc.scalar.activation(
                            aggT[:, b * P:b * P + rb], pg[:, :rb],
                            mybir.ActivationFunctionType.Relu,
                            bias=bc[:, layer:layer + 1])
                        if w_next is not None and ((b + 1) % 4 == 0 or b == nb - 1):
                            wb0 = (b // 4) * WW
                            wcols = min(WW, sh - wb0)
                            ph = pwm.tile([P, WW], f32, tag="ph")
                            nc.tensor.matmul(ph[:, :wcols], w_next[:],
                                             aggT[:, wb0:wb0 + wcols],
                                             start=True, stop=True)
                            for j in range(0, wcols, P):
                                rb2 = min(P, wcols - j)
                                emit_shard_block(ph[:, j:j + rb2],
                                                 (wb0 + j) // P, rb2)
                    else:
                        # z block: bias, transpose into shard_sb (node-major)
                        zt = wp.tile([P, P], bf16, tag="zt")
                        nc.vector.tensor_scalar_add(zt[:, :rb], pg[:, :rb],
                                                    bc[:, layer:layer + 1])
                        pt = ptr.tile([P, P], bf16, tag="pt")
                        nc.tensor.transpose(pt[:rb, :], zt[:, :rb], ident[:])
                        nc.vector.tensor_copy(shard_sb[:rb, b, :], pt[:rb, :])
                if not relu:
                    flush_shard(3)
                elif w_next is not None:
                    flush_shard(next_layer)

            def do_weight_matmul(w, layer):
                for wb in range(0, sh, WW):
                    wcols = min(WW, sh - wb)
                    ph = pwm.tile([P, WW], f32, tag="ph")
                    nc.tensor.matmul(ph[:, :wcols], w[:], aggT[:, wb:wb + wcols],
                                     start=True, stop=True)
                    for j in range(0, wcols, P):
                        rb = min(P, wcols - j)
                        emit_shard_block(ph[:, j:j + rb], (wb + j) // P, rb)
                flush_shard(layer)

            stage = int(os.environ.get("GCN_STAGE", "5"))
            nc.gpsimd.memset(logits_sb[:], 0.0)
            if stage >= 1:
                do_allgather(0)
            if stage >= 2:
                do_aggregation(0, relu=True, w_next=w1, next_layer=1)
            if stage >= 3:
                do_allgather(1)
                do_aggregation(1, relu=True, w_next=w2, next_layer=2)
            if stage >= 4:
                do_allgather(2)
                do_aggregation(2, relu=False)   # writes z -> shard 3
                do_allgather(3)

            if stage >= 5:
                # ---- decode ----
                for k, (lo, hi, gr) in enumerate(dbatches):
                    w_ = hi - lo
                    za = gather_batch(dp, "za", full[3], idxA, k, lo, hi, gr >> 1)
                    zb = gather_batch(dp, "zb", full[3], idxB, k, lo, hi, gr & 1)
                    prod = dp.tile([P, w_, P], bf16, tag="prod")
                    nc.vector.tensor_tensor(
                        out=prod[:], in0=za[:], in1=zb[:],
                        op=mybir.AluOpType.mult)
                    nc.vector.tensor_reduce(
                        out=logits_sb[:, lo:hi], in_=prod[:],
                        axis=mybir.AxisListType.X, op=mybir.AluOpType.add)
            nc.sync.dma_start(logits_out[:], logits_sb[:])

    nc.compile()
    return nc


def _run(x, edge_index, edge_label_index, W0, b0, W1, b1, W2, b2):
    n, f_in = x.shape
    sh = n // NC
    lsh = edge_label_index.shape[1] // NC
    deg = np.bincount(edge_index[1].astype(np.int64), minlength=n).astype(np.float64) + 1.0
    dinv = (1.0 / np.sqrt(deg)).astype(np.float32)

    meta = _build_plan(n, edge_index, dinv)
    decm = _build_decode_plan(edge_label_index, lsh, sh, meta["nb"], meta["phalf"])
    nc = _build_bass(n, f_in, meta, decm)

    eye = np.eye(P, dtype=ml_dtypes.bfloat16)
    bcol = np.stack([b0, b1, b2], axis=1).astype(np.float32)
    nb = meta["nb"]
    dvb = np.zeros((NC, P, nb), np.float32)
    for c in range(NC):
        d = dinv[c * sh:(c + 1) * sh]
        d = np.pad(d, (0, nb * P - sh))
        dvb[c] = d.reshape(nb, P).T
    KIN = f_in // P
    xdt = ml_dtypes.bfloat16 if XBF16 else np.float32
    wdt = ml_dtypes.bfloat16 if WBF16 else np.float32

    diags = np.zeros((NC, P, nb * P), np.float32)
    for c in range(NC):
        for b in range(nb):
            np.fill_diagonal(diags[c, :, b * P:(b + 1) * P], dvb[c, :, b])
    diags = diags.astype(ml_dtypes.bfloat16)

    in_maps = []
    for c in range(NC):
        xs = x[c * sh:(c + 1) * sh].astype(np.float32)
        xT = np.ascontiguousarray(xs.T.reshape(KIN, P, sh)).astype(xdt)
        in_maps.append({
            "xT": xT,
            "W0": np.ascontiguousarray(W0.reshape(KIN, P, P)).astype(xdt),
            "W1": W1.astype(wdt), "W2": W2.astype(wdt),
            "bcols": bcol, "dinv_blk": dvb[c], "ident": eye,
            "ind": np.ascontiguousarray(meta["ind"][c]),
            "diag": np.ascontiguousarray(diags[c]),
            "agg_idx": np.ascontiguousarray(meta["agg_idx"][c]),
            "idxA": np.ascontiguousarray(decm["idxA"][c]),
            "idxB": np.ascontiguousarray(decm["idxB"][c]),
        })

    res = run_bass_kernel_spmd(nc, in_maps, core_ids=list(range(NC)),
                               trace=bool(os.environ.get("GCN_TRACE")))
    slot_lbl = decm["slot_lbl"]
    logits = np.zeros(NC * lsh, np.float32)
    for c in range(NC):
        lg = np.asarray(res.results[c]["logits"])   # [P, ldch], slot s=ch*P+p
        flat = lg.T.reshape(-1)
        valid = slot_lbl[c] >= 0
        logits[c * lsh + slot_lbl[c][valid]] = flat[valid]
    return logits, res


def kernel(x, edge_index, edge_label_index, W0, b0, W1, b1, W2, b2):
    logits, _ = _run(np.asarray(x), np.asarray(edge_index), np.asarray(edge_label_index),
                     np.asarray(W0), np.asarray(b0), np.asarray(W1), np.asarray(b1),
                     np.asarray(W2), np.asarray(b2))
    return logits
